# revision 1
# baseline (speedup 1.0000x reference)
"""Trainium2 Bass kernel for Conv2DCollapse_w_pillar (pillar scatter -> dense BEV).

Strategy ("one-hot matmul scatter"), data-parallel over batch (1 batch / core):
  - Host: dedup pillar rows per flat cell (last write wins, matching the
    reference), sort by cell, bucket into 256-cell blocks, pad each block to K
    rows.  Features are rounded to a single bf16 plane (rel err ~1e-3, well
    under the 2e-2 gate), packed contiguously as fe/fo = even/odd block rows.
  - Device: features upload as two contiguous DMAs into one [128, NPAIR*C]
    SBUF tile (even rows on partitions 0:K, odd on 64:64+K — engine APs need
    32-aligned partition starts).  DVE expands each chunk into the
    block-diagonal stationary layout [128, 64 pairs x 128] (even rows ->
    cols 0:64 of each 128-lane, odd rows -> cols 64:128).
    For each pair of blocks, DVE builds a one-hot oh[i, j] = (cell_id[i]==j),
    one bf16 matmul scatters+transposes the pair into PSUM (128 partitions =
    2 blocks x 64 channels).  ACT drains PSUM to SBUF, big DMAs write the
    dense (C, ny*nx) plane.  Every output element is written exactly once;
    empty cells get 0 from all-zero one-hot columns.
"""
import sys
sys.path.insert(0, "/opt/trn_rl_repo")
import numpy as np
import ml_dtypes

BF = ml_dtypes.bfloat16
NCORES = 8
C = 64
NX = 512
NY = 512
NXY = NX * NY
BC = 256                 # cells per block
NBLK = NXY // BC         # 1024 blocks per core
NPAIR = NBLK // 2        # 512 pairs per core
CHUNK_PAIRS = 32         # pairs per feature-expansion chunk
NCHUNK = NPAIR // CHUNK_PAIRS
GRP = 8                  # pairs per PSUM group (4 banks)
P = 128                  # partition rows: even rows at 0:K, odd at 64:64+K

_cache = {}


def _build_nc(K, LEAD=0, OUTB_BUFS=12, DRAIN_PAT="m4s2", OH_POOL="p8_136",
              CHUNK=32, MEMSET_MODE="full32", OH_RING=48,
              KE=None, KO=None, OG=2):
    import concourse.bass as bass
    import concourse.tile as tile
    from concourse import bacc, mybir
    from contextlib import ExitStack

    def drain_on_dve(gidx):
        if DRAIN_PAT == "m4":
            return gidx % 4 == 0
        if DRAIN_PAT == "m4s1":
            return gidx % 4 == 1
        if DRAIN_PAT == "m4s2":
            return gidx % 4 == 2
        if DRAIN_PAT == "m4s3":
            return gidx % 4 == 3
        if DRAIN_PAT == "m8":
            return gidx % 8 in (0, 4)
        if DRAIN_PAT == "m8b":
            return gidx % 8 in (1, 5)
        if DRAIN_PAT == "m5":
            return gidx % 5 == 0
        if DRAIN_PAT == "s4":
            return (gidx - 1) % 4 == 0
        if DRAIN_PAT == "m3":
            return gidx % 3 == 0
        if DRAIN_PAT == "m9":
            return gidx % 9 in (4, 8)
        raise ValueError(DRAIN_PAT)

    def oh_on_pool(p):
        if OH_POOL == "none":
            return False
        if OH_POOL == "p5_24":
            return p % 5 in (2, 4)
        if OH_POOL == "p8_257":
            return p % 8 in (2, 5, 7)
        if OH_POOL == "p8_146":
            return p % 8 in (1, 4, 6)
        if OH_POOL == "p8_036":
            return p % 8 in (0, 3, 6)
        if OH_POOL == "p8_136":
            return p % 8 in (1, 3, 6)
        if OH_POOL == "p3_1":
            return p % 3 == 1
        if OH_POOL == "p2_1":
            return p % 2 == 1
        raise ValueError(OH_POOL)

    dt = mybir.dt
    nchunk = NPAIR // CHUNK
    if KE is None:
        KE = (K,) * nchunk
    if KO is None:
        KO = (K,) * nchunk
    assert max(max(KE), max(KO)) <= K and len(KE) == len(KO) == nchunk
    KP = 64 + K
    assert K <= 64
    nc = bacc.Bacc("TRN2", target_bir_lowering=False, debug=False,
                   num_devices=NCORES)
    # packed features, per-chunk row heights stacked along dim 0
    fe = nc.dram_tensor("fe", [sum(KE), CHUNK * C], dt.bfloat16,
                        kind="ExternalInput").ap()
    fo = nc.dram_tensor("fo", [sum(KO), CHUNK * C], dt.bfloat16,
                        kind="ExternalInput").ap()
    cells_d = nc.dram_tensor("cells", [P, NPAIR], dt.float32,
                             kind="ExternalInput").ap()
    iota_d = nc.dram_tensor("iota", [P, BC], dt.bfloat16,
                            kind="ExternalInput").ap()
    zeros_d = None
    if K < 64:
        zeros_d = nc.dram_tensor("zeros", [64 - K, CHUNK * 128], dt.bfloat16,
                                 kind="ExternalInput").ap()
    # bf16 output: every value is exactly bf16-representable (features were
    # bf16-rounded; each output cell is a single such value or 0), so the
    # f32->bf16 drain cast and the host bf16->f32 upcast are both exact.
    # Halves the dominant output DMA traffic.
    out_d = nc.dram_tensor("out", [C, NXY], dt.bfloat16,
                           kind="ExternalOutput").ap()

    with tile.TileContext(nc) as tc, ExitStack() as ctx:
        const = ctx.enter_context(tc.tile_pool(name="const", bufs=1))
        featp = ctx.enter_context(tc.tile_pool(name="feat", bufs=1))
        lhsp = ctx.enter_context(tc.tile_pool(name="lhs", bufs=1))
        ohp = ctx.enter_context(tc.tile_pool(name="oh", bufs=1))
        outp = ctx.enter_context(tc.tile_pool(name="outb", bufs=OUTB_BUFS))
        psp = ctx.enter_context(tc.tile_pool(name="ps", bufs=2, space="PSUM"))

        iota_t = const.tile([P, BC], dt.bfloat16)
        cells_t = const.tile([P, NPAIR], dt.float32)

        # packed features: partitions 0:K = even rows, 64:64+K = odd rows
        fb = featp.tile([P, NPAIR * C], dt.bfloat16, tag="fb", name="fb")
        fb3 = fb[:].rearrange("k (p f) -> k p f", f=C)

        # persistent stationary tiles (block-diagonal layout), double-buffered
        NLHS = 2
        lhs = [lhsp.tile([P, CHUNK * 128], dt.bfloat16,
                         tag=f"lhs{b}", name=f"lhs{b}") for b in range(NLHS)]
        # zero the halves the expansion copies never write.  Rows K:64 /
        # 112:128 have cells=-1 -> all-zero one-hot, BUT uninitialized SBUF
        # can hold NaN bit patterns and 0*NaN = NaN, so those rows must be
        # zeroed too (full32 mode zeroes rows 32:64 / 96:128 full-width
        # before the expansions overwrite the live parts).
        z0 = lhs[0][:].rearrange("k (p f) -> k p f", f=128)
        z1 = lhs[1][:].rearrange("k (p f) -> k p f", f=128)
        if MEMSET_MODE == "whole":
            nc.vector.memset(lhs[0][:], 0.0)
            nc.scalar.memzero(lhs[1][:])
        elif MEMSET_MODE == "split3":
            # spread the 4 strided memsets: ACT+DVE for buf0, Pool for buf1
            nc.scalar.memzero(z0[0:K, :, C:128])
            nc.vector.memset(z0[64:64 + K, :, 0:C], 0.0)
            nc.gpsimd.memset(z1[0:K, :, C:128], 0.0)
            nc.gpsimd.memset(z1[64:64 + K, :, 0:C], 0.0)
            if K < 64:
                nc.sync.dma_start(lhs[0][K:64, :], zeros_d[:])
                nc.sync.dma_start(lhs[1][K:64, :], zeros_d[:])
        else:
            assert MEMSET_MODE == "full32"
            # matmuls contract over partitions [0:KP] only (KP = 64+K), so
            # rows 112:128 never feed the PE; rows K:64 are zeroed via cheap
            # DMA writes (no engine time); the never-written column halves
            # via strided memsets (buf0 DVE so chunk-0 expansion starts
            # early; buf1 Pool).
            nc.scalar.memzero(z0[0:K, :, C:128])
            nc.scalar.memzero(z0[64:64 + K, :, 0:C])
            nc.gpsimd.memset(z1[0:K, :, C:128], 0.0)
            nc.gpsimd.memset(z1[64:64 + K, :, 0:C], 0.0)
            if K < 64:
                nc.sync.dma_start(lhs[0][K:64, :], zeros_d[:])
                nc.sync.dma_start(lhs[1][K:64, :], zeros_d[:])

        rse = rso = 0
        for c in range(nchunk):
            p0 = c * CHUNK
            # feature upload, chunk-granular so chunk 0 compute starts early;
            # chunk 0 goes first in the queue, then the small constants
            nc.sync.dma_start(
                fb3[0:KE[c], p0:p0 + CHUNK, :],
                fe[rse:rse + KE[c], :].rearrange("k (p f) -> k p f", f=C))
            nc.sync.dma_start(
                fb3[64:64 + KO[c], p0:p0 + CHUNK, :],
                fo[rso:rso + KO[c], :].rearrange("k (p f) -> k p f", f=C))
            rse += KE[c]
            rso += KO[c]
            if c == 0:
                nc.sync.dma_start(iota_t[:], iota_d[:])
                nc.sync.dma_start(cells_t[:], cells_d[:])

        # software-pipelined emission: one-hots run LEAD pairs ahead of
        # their matmuls so a drain/expansion on DVE never starves PE
        oh_tiles = {}

        def emit_oh(p):
            oh = ohp.tile([P, BC], dt.bfloat16, tag=f"oh{p % OH_RING}",
                          name=f"oh{p % OH_RING}")
            oh_eng = nc.gpsimd if oh_on_pool(p) else nc.vector
            oh_eng.tensor_scalar(
                oh[0:KP, :], iota_t[0:KP, :], cells_t[0:KP, p:p + 1], None,
                mybir.AluOpType.is_equal)
            oh_tiles[p] = oh

        gidx = 0
        outb = None
        ps_t = None
        for step in range(NPAIR + LEAD):
            if step < NPAIR:
                emit_oh(step)
            q = step - LEAD
            if q < 0:
                continue
            c, qr = divmod(q, CHUNK)
            buf = c % NLHS
            p0 = c * CHUNK
            if qr == 0:
                # expand packed chunk into block-diagonal stationary layout
                # (DVE: gets the 4x copy mode, ~1.1us each)
                t3 = lhs[buf][:].rearrange("k (p f) -> k p f", f=128)
                nc.vector.tensor_copy(
                    t3[0:KE[c], :, 0:C],
                    fb3[0:KE[c], p0:p0 + CHUNK, :])
                nc.vector.tensor_copy(
                    t3[64:64 + KO[c], :, C:128],
                    fb3[64:64 + KO[c], p0:p0 + CHUNK, :])
            g, i = divmod(qr, GRP)
            if i == 0:
                if g % OG == 0:
                    outb = outp.tile([128, OG * GRP * BC], dt.bfloat16)
                ps_t = psp.tile([128, GRP * BC], dt.float32)
            sl = g * GRP + i
            nc.tensor.matmul(
                ps_t[:, i * BC:(i + 1) * BC],
                lhs[buf][0:KP, sl * 128:(sl + 1) * 128],
                oh_tiles.pop(q)[0:KP, :],
                start=True, stop=True)
            if i == GRP - 1:
                half = (g % OG) * GRP * BC
                # drain PSUM -> SBUF with exact f32->bf16 cast; mostly ACT
                gidx += 1
                if drain_on_dve(gidx):
                    nc.vector.tensor_copy(outb[:, half:half + GRP * BC], ps_t[:])
                else:
                    nc.scalar.copy(outb[:, half:half + GRP * BC], ps_t[:])
                if g % OG == OG - 1:
                    base = (p0 + (g - OG + 1) * GRP) * 2 * BC
                    dst4 = out_d[:, base:base + OG * GRP * 2 * BC].rearrange(
                        "c (p q r) -> c p q r", p=OG * GRP, q=2, r=BC)
                    src_e = outb[0:C, :].rearrange("c (p r) -> c p r", r=BC)
                    src_o = outb[C:128, :].rearrange("c (p r) -> c p r", r=BC)
                    nc.sync.dma_start(dst4[:, :, 0, :], src_e)
                    nc.sync.dma_start(dst4[:, :, 1, :], src_o)
    nc.compile()
    return nc


def _prep_core(pf, cell, K, KE, KO):
    """pf: (Nb, C) f32 features for this batch (deduped, sorted by cell);
    cell: (Nb,) int cell ids.  KE/KO: per-chunk padded row heights."""
    n = len(cell)
    block = cell // BC
    local = (cell % BC).astype(np.float32)
    starts = np.searchsorted(block, np.arange(NBLK))
    k = np.arange(n) - starts[block]
    assert k.max(initial=0) < K
    pair = block // 2
    parity = block % 2

    feat = pf.astype(BF)

    ev = parity == 0
    od = ~ev
    fe = np.zeros((K, NPAIR, C), dtype=BF)
    fo = np.zeros((K, NPAIR, C), dtype=BF)
    fe[k[ev], pair[ev], :] = feat[ev]
    fo[k[od], pair[od], :] = feat[od]
    # stack per-chunk trimmed slabs along rows (device reads them back
    # chunk by chunk)
    CH = CHUNK_PAIRS
    fe_p = np.concatenate(
        [fe[:KE[c], c * CH:(c + 1) * CH, :].reshape(KE[c], CH * C)
         for c in range(len(KE))], axis=0)
    fo_p = np.concatenate(
        [fo[:KO[c], c * CH:(c + 1) * CH, :].reshape(KO[c], CH * C)
         for c in range(len(KO))], axis=0)
    cells = np.full((P, NPAIR), -1.0, np.float32)
    cells[k[ev], pair[ev]] = local[ev]
    cells[64 + k[od], pair[od]] = local[od]
    m = {
        "fe": np.ascontiguousarray(fe_p),
        "fo": np.ascontiguousarray(fo_p),
        "cells": cells,
        "iota": np.broadcast_to(
            np.arange(BC, dtype=np.float32), (P, BC)).astype(BF).copy(),
    }
    if K < 64:
        m["zeros"] = np.zeros((64 - K, CHUNK_PAIRS * 128), dtype=BF)
    return m


def kernel(pillar_features, coords, batch_size, nx, ny, num_bev_features,
           **_ignored):
    from concourse import bass_utils

    pf = np.ascontiguousarray(np.asarray(pillar_features, dtype=np.float32))
    co = np.asarray(coords).astype(np.int64)
    B = int(batch_size)
    nx_i, ny_i, C_i = int(nx), int(ny), int(num_bev_features)
    assert (B, nx_i, ny_i, C_i) == (NCORES, NX, NY, C), "hardcoded shape mismatch"

    key = co[:, 0] * NXY + co[:, 1] + co[:, 2] * NX + co[:, 3]
    # dedup, last occurrence wins (matches reference .at[].set semantics)
    n = len(key)
    u, first_rev = np.unique(key[::-1], return_index=True)
    src = n - 1 - first_rev           # original row index that survives
    # u is sorted by (batch, cell)
    batch = (u // NXY).astype(np.int64)
    cell = (u % NXY).astype(np.int64)
    bstart = np.searchsorted(batch, np.arange(NCORES + 1))

    # K: max rows in any 256-cell block, rounded up (shared by all cores)
    blk_global = u // BC
    Kmax = int(np.max(np.bincount(blk_global, minlength=1))) if len(u) else 1
    K = max(8, -(-Kmax // 8) * 8)
    assert K <= 64, f"block occupancy {Kmax} too high for pair kernel"

    # per-chunk padded heights (max over cores; chunks 0/1 forced to K so
    # both lhs buffers are fully initialized on first use -- later chunks
    # leave stale-but-finite rows that all-zero one-hot columns ignore)
    CH = CHUNK_PAIRS
    nchunk = NPAIR // CH
    KEa = np.full(nchunk, 8, np.int64)
    KOa = np.full(nchunk, 8, np.int64)
    for b in range(NCORES):
        occ = np.bincount(cell[bstart[b]:bstart[b + 1]] // BC,
                          minlength=NBLK)
        KEa = np.maximum(KEa, occ[0::2].reshape(nchunk, CH).max(axis=1))
        KOa = np.maximum(KOa, occ[1::2].reshape(nchunk, CH).max(axis=1))
    KEa = np.minimum(-(-KEa // 8) * 8, K)
    KOa = np.minimum(-(-KOa // 8) * 8, K)
    KEa[:2] = K
    KOa[:2] = K
    KE = tuple(int(x) for x in KEa)
    KO = tuple(int(x) for x in KOa)

    import os as _os
    _knobs = {}
    for _k in ("LEAD", "OUTB_BUFS"):
        if _os.environ.get(f"KN_{_k}"):
            _knobs[_k] = int(_os.environ[f"KN_{_k}"])
    for _k in ("DRAIN_PAT", "OH_POOL", "MEMSET_MODE"):
        if _os.environ.get(f"KN_{_k}"):
            _knobs[_k] = _os.environ[f"KN_{_k}"]
    _key = (K, KE, KO) if not _knobs \
        else (K, KE, KO, tuple(sorted(_knobs.items())))
    if _key not in _cache:
        _cache[_key] = _build_nc(K, KE=KE, KO=KO, **_knobs)
    nc = _cache[_key]

    in_maps = []
    for b in range(NCORES):
        lo_i, hi_i = bstart[b], bstart[b + 1]
        in_maps.append(_prep_core(pf[src[lo_i:hi_i]], cell[lo_i:hi_i],
                                  K, KE, KO))

    import os
    trace = bool(os.environ.get("BASS_TRACE"))
    res = bass_utils.run_bass_kernel_spmd(
        nc, in_maps, core_ids=list(range(NCORES)), trace=trace)
    kernel._last_results = res

    out = np.empty((NCORES, C, NY, NX), dtype=np.float32)
    for b in range(NCORES):
        out[b] = np.asarray(res.results[b]["out"]).astype(
            np.float32).reshape(C, NY, NX)
    return out



# revision 3
# speedup vs baseline: 1.0326x; 1.0326x over previous
"""Trainium2 Bass kernel for Conv2DCollapse_w_pillar (pillar scatter -> dense BEV).

One-hot matmul scatter, data-parallel over batch (1 batch / core).  v2:
  - Input upload coalesced into per-group DMAs (uniform row height within a
    group) split across the SP and ACT HWDGE queues so the DMA device never
    idles waiting on issue (HWDGE is 625ns/DMA; 36 chunked DMAs used to gate
    the whole input phase).
  - cells table uploaded as bf16 (integers <= 256 are exact).
  - Drains (PSUM f32 -> SBUF bf16 cast) spread over ACT + DVE + Pool by a
    pattern knob; one-hots spread over DVE + Pool.  Engine cost model:
    drain [128, 2048] = 1858ns ACT / 2258ns DVE / 2939ns Pool; one-hot
    [KP, 256] = 127ns DVE / 451ns Pool.
  - Deeper outb backlog (SBUF freed by smaller oh ring + bf16 cells).
"""
import sys
sys.path.insert(0, "/opt/trn_rl_repo")
import numpy as np
import ml_dtypes

BF = ml_dtypes.bfloat16
NCORES = 8
C = 64
NX = 512
NY = 512
NXY = NX * NY
BC = 256                 # cells per block
NBLK = NXY // BC         # 1024 blocks per core
NPAIR = NBLK // 2        # 512 pairs per core
CHUNK = 32               # pairs per feature-expansion chunk
NCHUNK = NPAIR // CHUNK
GRP = 4                  # pairs per PSUM group (2 banks)
P = 128                  # partition rows: even rows at 0:K, odd at 64:64+K
OG = 4                   # PSUM groups per outb DMA
# input DMA groups: chunk ranges with uniform row height per group
GROUPS = ((0, 1), (1, 2), (2, 4), (4, 8), (8, 12), (12, 16))

_cache = {}


def _build_nc(K, KE, KO, LEAD=24, EXLEAD=16, OUTB_BUFS=10,
              DRAIN_PAT="ADAAADAAADAAADAA", OH_PAT="DDPDDPDDPDDDPDDP", OH_RING=48, SWAP=0,
              PREWAIT=0):
    """KE/KO: per-GROUP padded row heights (len == len(GROUPS))."""
    import concourse.bass as bass
    import concourse.tile as tile
    from concourse import bacc, mybir
    from contextlib import ExitStack

    dt = mybir.dt
    assert len(KE) == len(KO) == len(GROUPS)
    assert max(max(KE), max(KO)) <= K <= 64
    KP = 64 + K
    nc = bacc.Bacc("TRN2", target_bir_lowering=False, debug=False,
                   num_devices=NCORES)
    fe_d, fo_d = [], []
    for gi, (lo, hi) in enumerate(GROUPS):
        span = hi - lo
        fe_d.append(nc.dram_tensor(f"fe{gi}", [KE[gi], span * CHUNK * C],
                                   dt.bfloat16, kind="ExternalInput").ap())
        fo_d.append(nc.dram_tensor(f"fo{gi}", [KO[gi], span * CHUNK * C],
                                   dt.bfloat16, kind="ExternalInput").ap())
    cells_d = nc.dram_tensor("cells", [P, NPAIR], dt.bfloat16,
                             kind="ExternalInput").ap()
    iota_d = nc.dram_tensor("iota", [P, BC], dt.bfloat16,
                            kind="ExternalInput").ap()
    zeros_d = None
    if K < 64:
        zeros_d = nc.dram_tensor("zeros", [64 - K, CHUNK * 128], dt.bfloat16,
                                 kind="ExternalInput").ap()
    # bf16 output: every value is exactly bf16-representable (features were
    # bf16-rounded; each output cell is a single such value or 0), so the
    # f32->bf16 drain cast and the host bf16->f32 upcast are both exact.
    out_d = nc.dram_tensor("out", [C, NXY], dt.bfloat16,
                           kind="ExternalOutput").ap()

    with tile.TileContext(nc) as tc, ExitStack() as ctx:
        const = ctx.enter_context(tc.tile_pool(name="const", bufs=1))
        featp = ctx.enter_context(tc.tile_pool(name="feat", bufs=1))
        lhsp = ctx.enter_context(tc.tile_pool(name="lhs", bufs=1))
        ohp = ctx.enter_context(tc.tile_pool(name="oh", bufs=1))
        outp = ctx.enter_context(tc.tile_pool(name="outb", bufs=OUTB_BUFS))
        psp = ctx.enter_context(tc.tile_pool(name="ps", bufs=1, space="PSUM"))

        iota_t = const.tile([P, BC], dt.bfloat16)
        cells_b = const.tile([P, NPAIR], dt.bfloat16)
        cells_t = const.tile([P, NPAIR], dt.float32)

        # packed features: partitions 0:K = even rows, 64:64+K = odd rows
        fb = featp.tile([P, NPAIR * C], dt.bfloat16, tag="fb", name="fb")
        fb3 = fb[:].rearrange("k (p f) -> k p f", f=C)

        # persistent stationary tiles (block-diagonal layout), double-buffered
        NLHS = 2
        lhs = [lhsp.tile([P, CHUNK * 128], dt.bfloat16,
                         tag=f"lhs{b}", name=f"lhs{b}") for b in range(NLHS)]
        z0 = lhs[0][:].rearrange("k (p f) -> k p f", f=128)
        z1 = lhs[1][:].rearrange("k (p f) -> k p f", f=128)

        # rows K:64 feed the PE (contraction is 0:KP) but are never written
        # by expansions: zero them via DMA (cheap).  The never-written column
        # halves of rows 0:K / 64:64+K get strided memsets (could hold NaN
        # bit patterns and 0*NaN = NaN): lhs0 halves on DVE (fast 4x, done
        # before cells arrive), lhs1 halves on ACT (idle until first drain).
        # Chunks 0/1 are forced to height K so the first expansion fully
        # initializes the live parts.
        if K < 64:
            nc.gpsimd.dma_start(lhs[0][K:64, :], zeros_d[:])
            nc.gpsimd.dma_start(lhs[1][K:64, :], zeros_d[:])
        nc.vector.memset(z0[0:K, :, C:128], 0.0)
        nc.vector.memset(z0[64:64 + K, :, 0:C], 0.0)
        nc.gpsimd.memset(z1[0:K, :, C:128], 0.0)
        nc.gpsimd.memset(z1[64:64 + K, :, 0:C], 0.0)

        # input upload: cells/iota first on SP (one-hot critical path), fe
        # groups on SP, fo groups on ACT.  All dep-free (fb persistent), so
        # no head-of-line risk on either queue.
        nc.sync.dma_start(cells_b[:], cells_d[:])
        nc.sync.dma_start(iota_t[:], iota_d[:])
        # bf16 holds the integer cell ids (and -1) exactly; upcast on ACT
        # (idle until the first drain) to the f32 the is_equal scalar needs
        nc.scalar.copy(cells_t[:], cells_b[:])
        for gi, (lo, hi) in enumerate(GROUPS):
            span = hi - lo
            nc.sync.dma_start(
                fb3[0:KE[gi], lo * CHUNK:hi * CHUNK, :],
                fe_d[gi][:].rearrange("k (p f) -> k p f", f=C))
            nc.scalar.dma_start(
                fb3[64:64 + KO[gi], lo * CHUNK:hi * CHUNK, :],
                fo_d[gi][:].rearrange("k (p f) -> k p f", f=C))

        # per-chunk heights (group height of the containing group)
        ke_c = [0] * NCHUNK
        ko_c = [0] * NCHUNK
        for gi, (lo, hi) in enumerate(GROUPS):
            for c in range(lo, hi):
                ke_c[c] = KE[gi]
                ko_c[c] = KO[gi]

        oh_tiles = {}

        def emit_oh(p):
            oh = ohp.tile([P, BC], dt.bfloat16, tag=f"oh{p % OH_RING}",
                          name=f"oh{p % OH_RING}")
            oh_eng = nc.gpsimd if OH_PAT[p % len(OH_PAT)] == "P" else nc.vector
            oh_eng.tensor_scalar(
                oh[0:KP, :], iota_t[0:KP, :], cells_t[0:KP, p:p + 1], None,
                mybir.AluOpType.is_equal)
            oh_tiles[p] = (oh, None)

        assert LEAD >= EXLEAD

        def emit_exp(c):
            # expand packed chunk into block-diagonal stationary layout
            # (DVE 4x copy mode, ~0.6us each); emitted EXLEAD steps before
            # the chunk's first matmul so it never sits between a blocked
            # drain and the one-hots the next matmul group needs
            buf = c % NLHS
            p0 = c * CHUNK
            t3 = lhs[buf][:].rearrange("k (p f) -> k p f", f=128)
            nc.vector.tensor_copy(
                t3[0:ke_c[c], :, 0:C],
                fb3[0:ke_c[c], p0:p0 + CHUNK, :])
            nc.vector.tensor_copy(
                t3[64:64 + ko_c[c], :, C:128],
                fb3[64:64 + ko_c[c], p0:p0 + CHUNK, :])

        NGRP = NPAIR // GRP
        PSBUFS = 4
        # Matmul-group emission order: a group whose PSUM buffer is freed by
        # a slow Pool drain (g = pool_g + PSBUFS) is swapped with its
        # successor, whose buffer was freed by a fast ACT drain.  PSUM tags
        # stay pinned to the ORIGINAL group index, so the swapped-early
        # successor proceeds immediately and the stalled group gets one
        # extra group of calendar slack before it must dispatch.
        order = list(range(NGRP))
        if SWAP:
            for g in range(PSBUFS, NGRP - 1):
                if (DRAIN_PAT[(g - PSBUFS) % len(DRAIN_PAT)] == "P"
                        and order[g] == g and order[g + 1] == g + 1):
                    order[g], order[g + 1] = order[g + 1], order[g]
        qseq = [g * GRP + i for g in order for i in range(GRP)]

        gq = {}    # group -> psum tile
        ob = {}    # outb index -> sbuf tile
        done = {}  # outb index -> drained group count
        for step in range(NPAIR + LEAD):
            if step < NPAIR:
                emit_oh(step)
            s = step - LEAD
            qe = s + EXLEAD
            if qe >= 0 and qe % CHUNK == 0 and qe // CHUNK < NCHUNK:
                emit_exp(qe // CHUNK)
            if s < 0:
                continue
            q = qseq[s]
            g, i = divmod(q, GRP)
            buf = (q // CHUNK) % NLHS
            ok = g // OG
            if i == 0:
                if ok not in ob:
                    ob[ok] = outp.tile([128, OG * GRP * BC], dt.bfloat16,
                                       name="outb")
                gq[g] = psp.tile([128, GRP * BC], dt.float32,
                                 tag=f"ps{g % PSBUFS}", name=f"ps{g % PSBUFS}")
            ps_t = gq[g]
            sl = q % CHUNK
            oh, j = oh_tiles.pop(q)
            rhs = (oh[0:KP, :] if j is None else
                   oh[0:KP, :].rearrange("k (c j) -> k j c", j=4)[:, j, :])
            nc.tensor.matmul(
                ps_t[:, i * BC:(i + 1) * BC],
                lhs[buf][0:KP, sl * 128:(sl + 1) * 128],
                rhs,
                start=True, stop=True)
            if i == GRP - 1:
                outb = ob[ok]
                half = (g % OG) * GRP * BC
                # drain PSUM -> SBUF with exact f32->bf16 cast
                de = DRAIN_PAT[g % len(DRAIN_PAT)]
                if de == "D":
                    nc.vector.tensor_copy(outb[:, half:half + GRP * BC], ps_t[:])
                elif de == "P":
                    nc.gpsimd.tensor_copy(outb[:, half:half + GRP * BC], ps_t[:])
                else:
                    nc.scalar.copy(outb[:, half:half + GRP * BC], ps_t[:])
                del gq[g]
                done[ok] = done.get(ok, 0) + 1
                if done[ok] == OG:
                    base = ok * OG * GRP * 2 * BC
                    dst4 = out_d[:, base:base + OG * GRP * 2 * BC].rearrange(
                        "c (p q r) -> c p q r", p=OG * GRP, q=2, r=BC)
                    src_e = outb[0:C, :].rearrange("c (p r) -> c p r", r=BC)
                    src_o = outb[C:128, :].rearrange("c (p r) -> c p r", r=BC)
                    nc.sync.dma_start(dst4[:, :, 0, :], src_e)
                    nc.sync.dma_start(dst4[:, :, 1, :], src_o)
                    del ob[ok], done[ok]
    nc.compile()
    return nc


def _prep_core(pf, cell, K, KE, KO):
    """pf: (Nb, C) f32 features for this batch (deduped, sorted by cell);
    cell: (Nb,) int cell ids.  KE/KO: per-group padded row heights."""
    n = len(cell)
    block = cell // BC
    local = (cell % BC).astype(np.float32)
    starts = np.searchsorted(block, np.arange(NBLK))
    k = np.arange(n) - starts[block]
    assert k.max(initial=0) < K
    pair = block // 2
    parity = block % 2

    feat = pf.astype(BF)

    ev = parity == 0
    od = ~ev
    fe = np.zeros((K, NPAIR, C), dtype=BF)
    fo = np.zeros((K, NPAIR, C), dtype=BF)
    fe[k[ev], pair[ev], :] = feat[ev]
    fo[k[od], pair[od], :] = feat[od]
    m = {}
    for gi, (lo, hi) in enumerate(GROUPS):
        span = hi - lo
        m[f"fe{gi}"] = np.ascontiguousarray(
            fe[:KE[gi], lo * CHUNK:hi * CHUNK, :].reshape(KE[gi],
                                                          span * CHUNK * C))
        m[f"fo{gi}"] = np.ascontiguousarray(
            fo[:KO[gi], lo * CHUNK:hi * CHUNK, :].reshape(KO[gi],
                                                          span * CHUNK * C))
    cells = np.full((P, NPAIR), -1.0, np.float32)
    cells[k[ev], pair[ev]] = local[ev]
    cells[64 + k[od], pair[od]] = local[od]
    m["cells"] = cells.astype(BF)
    m["iota"] = np.broadcast_to(
        np.arange(BC, dtype=np.float32), (P, BC)).astype(BF).copy()
    if K < 64:
        m["zeros"] = np.zeros((64 - K, CHUNK * 128), dtype=BF)
    if K < 64:
        m["zeros"] = np.zeros((64 - K, CHUNK * 128), dtype=BF)
    return m


def _plan(cell_by_core):
    """Compute K and per-group heights from per-core cell id arrays."""
    nchunk = NCHUNK
    KEa = np.zeros(nchunk, np.int64)
    KOa = np.zeros(nchunk, np.int64)
    for cells in cell_by_core:
        occ = np.bincount(cells // BC, minlength=NBLK)
        KEa = np.maximum(KEa, occ[0::2].reshape(nchunk, CHUNK).max(axis=1))
        KOa = np.maximum(KOa, occ[1::2].reshape(nchunk, CHUNK).max(axis=1))
    K = int(max(KEa.max(), KOa.max(), 8))
    KE, KO = [], []
    for gi, (lo, hi) in enumerate(GROUPS):
        ke = int(KEa[lo:hi].max())
        ko = int(KOa[lo:hi].max())
        # chunks 0 and 1 (the first use of each lhs buffer) must fully
        # initialize rows 0:K / 64:64+K
        if lo <= 0 < hi or lo <= 1 < hi:
            ke = ko = K
        KE.append(ke)
        KO.append(ko)
    return K, tuple(KE), tuple(KO)


def kernel(pillar_features, coords, batch_size, nx, ny, num_bev_features,
           **_ignored):
    from concourse import bass_utils

    pf = np.ascontiguousarray(np.asarray(pillar_features, dtype=np.float32))
    co = np.asarray(coords).astype(np.int64)
    B = int(batch_size)
    nx_i, ny_i, C_i = int(nx), int(ny), int(num_bev_features)
    assert (B, nx_i, ny_i, C_i) == (NCORES, NX, NY, C), "hardcoded shape mismatch"

    key = co[:, 0] * NXY + co[:, 1] + co[:, 2] * NX + co[:, 3]
    # dedup, last occurrence wins (matches reference .at[].set semantics)
    n = len(key)
    u, first_rev = np.unique(key[::-1], return_index=True)
    src = n - 1 - first_rev           # original row index that survives
    batch = (u // NXY).astype(np.int64)
    cell = (u % NXY).astype(np.int64)
    bstart = np.searchsorted(batch, np.arange(NCORES + 1))

    cell_by_core = [cell[bstart[b]:bstart[b + 1]] for b in range(NCORES)]
    K, KE, KO = _plan(cell_by_core)

    import os as _os
    _knobs = {}
    for _k in ("LEAD", "EXLEAD", "OUTB_BUFS", "OH_RING", "SWAP", "PREWAIT"):
        if _os.environ.get(f"KN_{_k}"):
            _knobs[_k] = int(_os.environ[f"KN_{_k}"])
    for _k in ("DRAIN_PAT", "OH_PAT"):
        if _os.environ.get(f"KN_{_k}"):
            _knobs[_k] = _os.environ[f"KN_{_k}"]
    _key = (K, KE, KO) if not _knobs \
        else (K, KE, KO, tuple(sorted(_knobs.items())))
    if _key not in _cache:
        _cache[_key] = _build_nc(K, KE, KO, **_knobs)
    nc = _cache[_key]

    in_maps = []
    for b in range(NCORES):
        lo_i, hi_i = bstart[b], bstart[b + 1]
        in_maps.append(_prep_core(pf[src[lo_i:hi_i]], cell_by_core[b],
                                  K, KE, KO))

    import os
    trace = bool(os.environ.get("BASS_TRACE"))
    res = bass_utils.run_bass_kernel_spmd(
        nc, in_maps, core_ids=list(range(NCORES)), trace=trace)
    kernel._last_results = res

    out = np.empty((NCORES, C, NY, NX), dtype=np.float32)
    for b in range(NCORES):
        out[b] = np.asarray(res.results[b]["out"]).astype(
            np.float32).reshape(C, NY, NX)
    return out


# revision 4
# speedup vs baseline: 1.0711x; 1.0373x over previous
"""Trainium2 Bass kernel for Conv2DCollapse_w_pillar (pillar scatter -> dense BEV).

One-hot matmul scatter, data-parallel over batch (1 batch / core).  v2:
  - Input upload coalesced into per-group DMAs (uniform row height within a
    group) split across the SP and ACT HWDGE queues so the DMA device never
    idles waiting on issue (HWDGE is 625ns/DMA; 36 chunked DMAs used to gate
    the whole input phase).
  - cells table uploaded as bf16 (integers <= 256 are exact).
  - Drains (PSUM f32 -> SBUF bf16 cast) spread over ACT + DVE + Pool by a
    pattern knob; one-hots spread over DVE + Pool.  Engine cost model:
    drain [128, 2048] = 1858ns ACT / 2258ns DVE / 2939ns Pool; one-hot
    [KP, 256] = 127ns DVE / 451ns Pool.
  - Deeper outb backlog (SBUF freed by smaller oh ring + bf16 cells).
"""
import sys
sys.path.insert(0, "/opt/trn_rl_repo")
import numpy as np
import ml_dtypes

BF = ml_dtypes.bfloat16
NCORES = 8
C = 64
NX = 512
NY = 512
NXY = NX * NY
BC = 256                 # cells per block
NBLK = NXY // BC         # 1024 blocks per core
NPAIR = NBLK // 2        # 512 pairs per core
CHUNK = 32               # pairs per feature-expansion chunk
NCHUNK = NPAIR // CHUNK
GRP = 4                  # pairs per PSUM group (2 banks)
P = 128                  # partition rows: even rows at 0:K, odd at 64:64+K
OG = 4                   # PSUM groups per outb DMA
# input DMA groups: chunk ranges with uniform row height per group
GROUPS = ((0, 1), (1, 2), (2, 4), (4, 8), (8, 12), (12, 16))

_cache = {}


def _build_nc(K, KE, KO, LEAD=8, EXLEAD=8, OUTB_BUFS=14,
              DRAIN_PAT="ADAAADAAADAAADAA", OH_PAT="DDPDDPDDPDDDPDDP",
              OH_RING=16, SWAP=0, PREWAIT=0):
    """KE/KO: per-GROUP padded row heights (len == len(GROUPS))."""
    import concourse.bass as bass
    import concourse.tile as tile
    from concourse import bacc, mybir
    from contextlib import ExitStack

    dt = mybir.dt
    assert len(KE) == len(KO) == len(GROUPS)
    assert max(max(KE), max(KO)) <= K <= 64
    KP = 64 + K
    nc = bacc.Bacc("TRN2", target_bir_lowering=False, debug=False,
                   num_devices=NCORES)
    fe_d, fo_d = [], []
    for gi, (lo, hi) in enumerate(GROUPS):
        span = hi - lo
        fe_d.append(nc.dram_tensor(f"fe{gi}", [KE[gi], span * CHUNK * C],
                                   dt.bfloat16, kind="ExternalInput").ap())
        fo_d.append(nc.dram_tensor(f"fo{gi}", [KO[gi], span * CHUNK * C],
                                   dt.bfloat16, kind="ExternalInput").ap())
    cells_d = nc.dram_tensor("cells", [P, NPAIR], dt.bfloat16,
                             kind="ExternalInput").ap()
    iota_d = nc.dram_tensor("iota", [P, BC], dt.bfloat16,
                            kind="ExternalInput").ap()
    zeros_d = None
    if K < 64:
        zeros_d = nc.dram_tensor("zeros", [64 - K, CHUNK * 128], dt.bfloat16,
                                 kind="ExternalInput").ap()
    # bf16 output: every value is exactly bf16-representable (features were
    # bf16-rounded; each output cell is a single such value or 0), so the
    # f32->bf16 drain cast and the host bf16->f32 upcast are both exact.
    out_d = nc.dram_tensor("out", [C, NXY], dt.bfloat16,
                           kind="ExternalOutput").ap()

    with tile.TileContext(nc) as tc, ExitStack() as ctx:
        const = ctx.enter_context(tc.tile_pool(name="const", bufs=1))
        featp = ctx.enter_context(tc.tile_pool(name="feat", bufs=1))
        lhsp = ctx.enter_context(tc.tile_pool(name="lhs", bufs=1))
        ohp = ctx.enter_context(tc.tile_pool(name="oh", bufs=1))
        outp = ctx.enter_context(tc.tile_pool(name="outb", bufs=OUTB_BUFS))
        psp = ctx.enter_context(tc.tile_pool(name="ps", bufs=1, space="PSUM"))

        iota_t = const.tile([P, BC], dt.bfloat16)
        cells_b = const.tile([P, NPAIR], dt.bfloat16)
        cells_t = const.tile([P, NPAIR], dt.float32)

        # packed features: partitions 0:K = even rows, 64:64+K = odd rows
        fb = featp.tile([P, NPAIR * C], dt.bfloat16, tag="fb", name="fb")
        fb3 = fb[:].rearrange("k (p f) -> k p f", f=C)

        # persistent stationary tiles (block-diagonal layout), double-buffered
        NLHS = 2
        lhs = [lhsp.tile([P, CHUNK * 128], dt.bfloat16,
                         tag=f"lhs{b}", name=f"lhs{b}") for b in range(NLHS)]
        z0 = lhs[0][:].rearrange("k (p f) -> k p f", f=128)
        z1 = lhs[1][:].rearrange("k (p f) -> k p f", f=128)

        # rows K:64 feed the PE (contraction is 0:KP) but are never written
        # by expansions: zero them via DMA (cheap).  The never-written column
        # halves of rows 0:K / 64:64+K get strided memsets (could hold NaN
        # bit patterns and 0*NaN = NaN): lhs0 halves on DVE (fast 4x, done
        # before cells arrive), lhs1 halves on ACT (idle until first drain).
        # Chunks 0/1 are forced to height K so the first expansion fully
        # initializes the live parts.
        if K < 64:
            nc.gpsimd.dma_start(lhs[0][K:64, :], zeros_d[:])
            nc.gpsimd.dma_start(lhs[1][K:64, :], zeros_d[:])
        nc.vector.memset(z0[0:K, :, C:128], 0.0)
        nc.vector.memset(z0[64:64 + K, :, 0:C], 0.0)
        nc.gpsimd.memset(z1[0:K, :, C:128], 0.0)
        nc.gpsimd.memset(z1[64:64 + K, :, 0:C], 0.0)

        # input upload: cells/iota first on SP (one-hot critical path), fe
        # groups on SP, fo groups on ACT.  All dep-free (fb persistent), so
        # no head-of-line risk on either queue.
        nc.sync.dma_start(cells_b[:], cells_d[:])
        nc.sync.dma_start(iota_t[:], iota_d[:])
        # bf16 holds the integer cell ids (and -1) exactly; upcast on ACT
        # (idle until the first drain) to the f32 the is_equal scalar needs
        nc.scalar.copy(cells_t[:], cells_b[:])
        for gi, (lo, hi) in enumerate(GROUPS):
            span = hi - lo
            nc.sync.dma_start(
                fb3[0:KE[gi], lo * CHUNK:hi * CHUNK, :],
                fe_d[gi][:].rearrange("k (p f) -> k p f", f=C))
            nc.scalar.dma_start(
                fb3[64:64 + KO[gi], lo * CHUNK:hi * CHUNK, :],
                fo_d[gi][:].rearrange("k (p f) -> k p f", f=C))

        # per-chunk heights (group height of the containing group)
        ke_c = [0] * NCHUNK
        ko_c = [0] * NCHUNK
        for gi, (lo, hi) in enumerate(GROUPS):
            for c in range(lo, hi):
                ke_c[c] = KE[gi]
                ko_c[c] = KO[gi]

        oh_tiles = {}

        def emit_oh(p):
            oh = ohp.tile([P, BC], dt.bfloat16, tag=f"oh{p % OH_RING}",
                          name=f"oh{p % OH_RING}")
            oh_eng = nc.gpsimd if OH_PAT[p % len(OH_PAT)] == "P" else nc.vector
            oh_eng.tensor_scalar(
                oh[0:KP, :], iota_t[0:KP, :], cells_t[0:KP, p:p + 1], None,
                mybir.AluOpType.is_equal)
            oh_tiles[p] = (oh, None)

        assert LEAD >= EXLEAD

        def emit_exp(c):
            # expand packed chunk into block-diagonal stationary layout
            # (DVE 4x copy mode, ~0.6us each); emitted EXLEAD steps before
            # the chunk's first matmul so it never sits between a blocked
            # drain and the one-hots the next matmul group needs
            buf = c % NLHS
            p0 = c * CHUNK
            t3 = lhs[buf][:].rearrange("k (p f) -> k p f", f=128)
            nc.vector.tensor_copy(
                t3[0:ke_c[c], :, 0:C],
                fb3[0:ke_c[c], p0:p0 + CHUNK, :])
            nc.vector.tensor_copy(
                t3[64:64 + ko_c[c], :, C:128],
                fb3[64:64 + ko_c[c], p0:p0 + CHUNK, :])

        NGRP = NPAIR // GRP
        PSBUFS = 4
        # Matmul-group emission order: a group whose PSUM buffer is freed by
        # a slow Pool drain (g = pool_g + PSBUFS) is swapped with its
        # successor, whose buffer was freed by a fast ACT drain.  PSUM tags
        # stay pinned to the ORIGINAL group index, so the swapped-early
        # successor proceeds immediately and the stalled group gets one
        # extra group of calendar slack before it must dispatch.
        order = list(range(NGRP))
        if SWAP:
            for g in range(PSBUFS, NGRP - 1):
                if (DRAIN_PAT[(g - PSBUFS) % len(DRAIN_PAT)] == "P"
                        and order[g] == g and order[g + 1] == g + 1):
                    order[g], order[g + 1] = order[g + 1], order[g]
        qseq = [g * GRP + i for g in order for i in range(GRP)]

        gq = {}    # group -> psum tile
        ob = {}    # outb index -> sbuf tile
        done = {}  # outb index -> drained group count
        for step in range(NPAIR + LEAD):
            if step < NPAIR:
                emit_oh(step)
            s = step - LEAD
            qe = s + EXLEAD
            if qe >= 0 and qe % CHUNK == 0 and qe // CHUNK < NCHUNK:
                emit_exp(qe // CHUNK)
            if s < 0:
                continue
            q = qseq[s]
            g, i = divmod(q, GRP)
            buf = (q // CHUNK) % NLHS
            ok = g // OG
            if i == 0:
                if ok not in ob:
                    ob[ok] = outp.tile([128, OG * GRP * BC], dt.bfloat16,
                                       name="outb")
                gq[g] = psp.tile([128, GRP * BC], dt.float32,
                                 tag=f"ps{g % PSBUFS}", name=f"ps{g % PSBUFS}")
            ps_t = gq[g]
            sl = q % CHUNK
            oh, j = oh_tiles.pop(q)
            rhs = (oh[0:KP, :] if j is None else
                   oh[0:KP, :].rearrange("k (c j) -> k j c", j=4)[:, j, :])
            nc.tensor.matmul(
                ps_t[:, i * BC:(i + 1) * BC],
                lhs[buf][0:KP, sl * 128:(sl + 1) * 128],
                rhs,
                start=True, stop=True)
            if i == GRP - 1:
                outb = ob[ok]
                half = (g % OG) * GRP * BC
                # drain PSUM -> SBUF with exact f32->bf16 cast
                de = DRAIN_PAT[g % len(DRAIN_PAT)]
                if de == "D":
                    nc.vector.tensor_copy(outb[:, half:half + GRP * BC], ps_t[:])
                elif de == "P":
                    nc.gpsimd.tensor_copy(outb[:, half:half + GRP * BC], ps_t[:])
                else:
                    nc.scalar.copy(outb[:, half:half + GRP * BC], ps_t[:])
                del gq[g]
                done[ok] = done.get(ok, 0) + 1
                if done[ok] == OG:
                    base = ok * OG * GRP * 2 * BC
                    dst4 = out_d[:, base:base + OG * GRP * 2 * BC].rearrange(
                        "c (p q r) -> c p q r", p=OG * GRP, q=2, r=BC)
                    src_e = outb[0:C, :].rearrange("c (p r) -> c p r", r=BC)
                    src_o = outb[C:128, :].rearrange("c (p r) -> c p r", r=BC)
                    nc.sync.dma_start(dst4[:, :, 0, :], src_e)
                    nc.sync.dma_start(dst4[:, :, 1, :], src_o)
                    del ob[ok], done[ok]
    nc.compile()
    return nc


def _prep_core(pf, cell, K, KE, KO):
    """pf: (Nb, C) f32 features for this batch (deduped, sorted by cell);
    cell: (Nb,) int cell ids.  KE/KO: per-group padded row heights."""
    n = len(cell)
    block = cell // BC
    local = (cell % BC).astype(np.float32)
    starts = np.searchsorted(block, np.arange(NBLK))
    k = np.arange(n) - starts[block]
    assert k.max(initial=0) < K
    pair = block // 2
    parity = block % 2

    feat = pf.astype(BF)

    ev = parity == 0
    od = ~ev
    fe = np.zeros((K, NPAIR, C), dtype=BF)
    fo = np.zeros((K, NPAIR, C), dtype=BF)
    fe[k[ev], pair[ev], :] = feat[ev]
    fo[k[od], pair[od], :] = feat[od]
    m = {}
    for gi, (lo, hi) in enumerate(GROUPS):
        span = hi - lo
        m[f"fe{gi}"] = np.ascontiguousarray(
            fe[:KE[gi], lo * CHUNK:hi * CHUNK, :].reshape(KE[gi],
                                                          span * CHUNK * C))
        m[f"fo{gi}"] = np.ascontiguousarray(
            fo[:KO[gi], lo * CHUNK:hi * CHUNK, :].reshape(KO[gi],
                                                          span * CHUNK * C))
    cells = np.full((P, NPAIR), -1.0, np.float32)
    cells[k[ev], pair[ev]] = local[ev]
    cells[64 + k[od], pair[od]] = local[od]
    m["cells"] = cells.astype(BF)
    m["iota"] = np.broadcast_to(
        np.arange(BC, dtype=np.float32), (P, BC)).astype(BF).copy()
    if K < 64:
        m["zeros"] = np.zeros((64 - K, CHUNK * 128), dtype=BF)
    if K < 64:
        m["zeros"] = np.zeros((64 - K, CHUNK * 128), dtype=BF)
    return m


def _plan(cell_by_core):
    """Compute K and per-group heights from per-core cell id arrays."""
    nchunk = NCHUNK
    KEa = np.zeros(nchunk, np.int64)
    KOa = np.zeros(nchunk, np.int64)
    for cells in cell_by_core:
        occ = np.bincount(cells // BC, minlength=NBLK)
        KEa = np.maximum(KEa, occ[0::2].reshape(nchunk, CHUNK).max(axis=1))
        KOa = np.maximum(KOa, occ[1::2].reshape(nchunk, CHUNK).max(axis=1))
    K = int(max(KEa.max(), KOa.max(), 8))
    KE, KO = [], []
    for gi, (lo, hi) in enumerate(GROUPS):
        ke = int(KEa[lo:hi].max())
        ko = int(KOa[lo:hi].max())
        # chunks 0 and 1 (the first use of each lhs buffer) must fully
        # initialize rows 0:K / 64:64+K
        if lo <= 0 < hi or lo <= 1 < hi:
            ke = ko = K
        KE.append(ke)
        KO.append(ko)
    return K, tuple(KE), tuple(KO)


def kernel(pillar_features, coords, batch_size, nx, ny, num_bev_features,
           **_ignored):
    from concourse import bass_utils

    pf = np.ascontiguousarray(np.asarray(pillar_features, dtype=np.float32))
    co = np.asarray(coords).astype(np.int64)
    B = int(batch_size)
    nx_i, ny_i, C_i = int(nx), int(ny), int(num_bev_features)
    assert (B, nx_i, ny_i, C_i) == (NCORES, NX, NY, C), "hardcoded shape mismatch"

    key = co[:, 0] * NXY + co[:, 1] + co[:, 2] * NX + co[:, 3]
    # dedup, last occurrence wins (matches reference .at[].set semantics)
    n = len(key)
    u, first_rev = np.unique(key[::-1], return_index=True)
    src = n - 1 - first_rev           # original row index that survives
    batch = (u // NXY).astype(np.int64)
    cell = (u % NXY).astype(np.int64)
    bstart = np.searchsorted(batch, np.arange(NCORES + 1))

    cell_by_core = [cell[bstart[b]:bstart[b + 1]] for b in range(NCORES)]
    K, KE, KO = _plan(cell_by_core)

    import os as _os
    _knobs = {}
    for _k in ("LEAD", "EXLEAD", "OUTB_BUFS", "OH_RING", "SWAP", "PREWAIT"):
        if _os.environ.get(f"KN_{_k}"):
            _knobs[_k] = int(_os.environ[f"KN_{_k}"])
    for _k in ("DRAIN_PAT", "OH_PAT"):
        if _os.environ.get(f"KN_{_k}"):
            _knobs[_k] = _os.environ[f"KN_{_k}"]
    _key = (K, KE, KO) if not _knobs \
        else (K, KE, KO, tuple(sorted(_knobs.items())))
    if _key not in _cache:
        _cache[_key] = _build_nc(K, KE, KO, **_knobs)
    nc = _cache[_key]

    in_maps = []
    for b in range(NCORES):
        lo_i, hi_i = bstart[b], bstart[b + 1]
        in_maps.append(_prep_core(pf[src[lo_i:hi_i]], cell_by_core[b],
                                  K, KE, KO))

    import os
    trace = bool(os.environ.get("BASS_TRACE"))
    res = bass_utils.run_bass_kernel_spmd(
        nc, in_maps, core_ids=list(range(NCORES)), trace=trace)
    kernel._last_results = res

    out = np.empty((NCORES, C, NY, NX), dtype=np.float32)
    for b in range(NCORES):
        out[b] = np.asarray(res.results[b]["out"]).astype(
            np.float32).reshape(C, NY, NX)
    return out
